# revision 1
# baseline (speedup 1.0000x reference)
"""BinaryLinear (sign-binarized weight linear layer) on 8 Trainium2 NeuronCores.

y[b,s,o] = sum_i x[b,s,i] * (scale[o] * sign(w[o,i])) + bias[o]
  with scale[o] = mean_i |w[o,i]|

Sharding: data-parallel over the batch dim (8 batches -> 8 cores). Each core:
  - casts its x shard f32->bf16 (SWDGE cast-DMA, DRAM->DRAM)
  - binarizes the (replicated) weight on-chip: ACT Sign -> bf16 B matrix,
    DVE abs-reduce -> per-row scale
  - computes yT = (B @ xT) * scale + bias with the production tile matmul
    (both operands fed K-major via bf16 XBAR DMA-transpose)
Host side only shards inputs and transposes yT shards back into y.
"""

import numpy as np

B_DIM = 8
S_DIM = 2048
IN_F = 4096
OUT_F = 4096
P = 128
N_CORES = 8

_BUILT = None


def _build_nc(s_dim=S_DIM, in_f=IN_F, out_f=OUT_F):
    from contextlib import ExitStack

    import concourse.bass as bass
    import concourse.mybir as mybir
    import concourse.tile as tile
    from concourse import bacc
    from concourse.bass import ts
    from concourse.kernels.tile_matmul import (
        composable_matmul_tile_kernel,
        dma_from_dram_kxm,
        dma_from_dram_kxn,
        dma_to_dram_mxn,
        k_pool_min_bufs,
    )

    f32 = mybir.dt.float32
    bf16 = mybir.dt.bfloat16

    nc = bacc.Bacc(None, target_bir_lowering=False, debug=False)
    with tile.TileContext(nc) as tc:
        x_d = nc.dram_tensor("x", (s_dim, in_f), f32, kind="ExternalInput")
        w_d = nc.dram_tensor("w", (out_f, in_f), f32, kind="ExternalInput")
        b_d = nc.dram_tensor("bias", (out_f,), f32, kind="ExternalInput")
        yT_d = nc.dram_tensor("yT", (out_f, s_dim), f32, kind="ExternalOutput")

        with ExitStack() as ctx:
            dram = ctx.enter_context(tc.tile_pool(name="dram", bufs=1, space="DRAM"))
            xbf_d = dram.tile((s_dim, in_f), bf16)
            bw_d = dram.tile((out_f, in_f), bf16)

            const = ctx.enter_context(tc.tile_pool(name="const", bufs=1))
            o_blocks = out_f // P
            scale_sb = const.tile([P, o_blocks], f32)
            bias_sb = const.tile([P, o_blocks], f32)
            nc.sync.dma_start(
                bias_sb[:], b_d[:].rearrange("(po pi) -> pi po", pi=P)
            )

            # ---- phase 1a: x f32 -> bf16 (cast during SWDGE DMA, DRAM->DRAM)
            CH = 512
            for i in range(s_dim // CH):
                nc.gpsimd.dma_start(xbf_d[ts(i, CH), :], x_d[ts(i, CH), :])

            # ---- phase 1b: binarize W, accumulate |w| row sums
            wpool = ctx.enter_context(tc.tile_pool(name="wpool", bufs=2))
            bpool = ctx.enter_context(tc.tile_pool(name="bpool", bufs=2))
            for po in range(o_blocks):
                w_sb = wpool.tile([P, in_f], f32)
                nc.sync.dma_start(w_sb[:], w_d[ts(po, P), :])
                b_sb = bpool.tile([P, in_f], bf16)
                nc.scalar.sign(b_sb[:], w_sb[:])
                nc.vector.tensor_reduce(
                    scale_sb[:, po : po + 1],
                    w_sb[:],
                    axis=mybir.AxisListType.X,
                    op=mybir.AluOpType.add,
                    apply_absolute_value=True,
                )
                nc.sync.dma_start(bw_d[ts(po, P), :], b_sb[:])
            nc.vector.tensor_scalar_mul(scale_sb[:], scale_sb[:], 1.0 / in_f)

            # ---- phase 2: yT[o, m] = B[o,:] . x[m,:] contracted over k
            # kxm = B^T (stationary side, streamed once), kxn = x^T (moving)
            bw_ap = bw_d[:, :]
            xbf_ap = xbf_d[:, :]
            kxm_pool = ctx.enter_context(
                tc.tile_pool(
                    name="kxm_pool",
                    bufs=k_pool_min_bufs(bw_ap, transpose_ap=True),
                )
            )
            kxn_pool = ctx.enter_context(
                tc.tile_pool(
                    name="kxn_pool",
                    bufs=k_pool_min_bufs(xbf_ap, transpose_ap=True),
                )
            )
            kxm_producer, kxm_shape = dma_from_dram_kxm(
                kxm_pool, bw_ap, transpose_ap=True
            )
            kxn_producer, kxn_shape = dma_from_dram_kxn(
                kxn_pool, xbf_ap, transpose_ap=True
            )
            mxn_consumer = dma_to_dram_mxn(yT_d[:, :])

            def scale_bias_reducer(nc, psum, sbuf, md):
                po_idx = md.m_tile_idx * md.m_subtiles + md.m_subtile_idx
                nc.vector.tensor_scalar(
                    sbuf[:, 0, : md.n_slice_size],
                    psum[:, : md.n_slice_size],
                    scale_sb[:, po_idx : po_idx + 1],
                    bias_sb[:, po_idx : po_idx + 1],
                    op0=mybir.AluOpType.mult,
                    op1=mybir.AluOpType.add,
                )

            composable_matmul_tile_kernel(
                tc,
                kxm_shape=kxm_shape,
                kxn_shape=kxn_shape,
                output_type=f32,
                kxm_producer=kxm_producer,
                kxn_producer=kxn_producer,
                mxn_consumer=mxn_consumer,
                mxn_subtile_reducer=scale_bias_reducer,
                cache_tiles=True,
                psum_n_bufs=2,
            )
    nc.finalize()
    return nc


def _get_nc():
    global _BUILT
    if _BUILT is None:
        _BUILT = _build_nc()
    return _BUILT


def kernel(x, weight, bias):
    from concourse.bass_utils import run_bass_kernel_spmd

    x = np.asarray(x, dtype=np.float32)
    weight = np.asarray(weight, dtype=np.float32)
    bias = np.asarray(bias, dtype=np.float32)
    assert x.shape == (B_DIM, S_DIM, IN_F), x.shape

    nc = _get_nc()
    in_maps = [
        {"x": np.ascontiguousarray(x[b]), "w": weight, "bias": bias}
        for b in range(N_CORES)
    ]
    res = run_bass_kernel_spmd(nc, in_maps, core_ids=list(range(N_CORES)))
    out = np.empty((B_DIM, S_DIM, OUT_F), dtype=np.float32)
    for b in range(N_CORES):
        out[b] = res.results[b]["yT"].T
    return out
